# revision 4
# baseline (speedup 1.0000x reference)
"""Block-circulant linear (MINI_BLOCK=4) via length-4 rFFT factorization on 8 trn2 cores.

Math: out = x @ W^T where W[4y+n, 4x+j] = eigens[y, x, (n-j) mod 4].
In the length-4 DFT domain the circulant contraction factors into 6 real
matmuls over the 1024-dim block-index axis (10.7x fewer FLOPs than dense):
  X0 = x0+x1+x2+x3, X1 = (x0-x2) + i(x3-x1), X2 = x0-x1+x2-x3  (per block of 4)
  Y0 = X0 E0, Y1 = X1*E1 (complex), Y2 = X2 E2   (contract over gx=1024)
  o0 = Y0+2Re(Y1)+Y2, o1 = Y0-2Im(Y1)-Y2, o2 = Y0-2Re(Y1)+Y2, o3 = Y0+2Im(Y1)-Y2 (/4)

Sharding: data-parallel over batch, 512 rows per core. E-matrices (host
pre-transformed from eigens, scale factors folded) are replicated per core.
Matmuls run in float32r (fp32 bits, reduced-precision PE multiply, ~4x faster
than fp32; measured rel err ~1.5e-4 over K=1024).
"""
import numpy as np

B, IN, OUT, BLK = 4096, 4096, 4096, 4
GX, GY = IN // BLK, OUT // BLK        # 1024, 1024
NCORES = 8
BS = B // NCORES                      # 512 batch rows per core
BT = BS // 128                        # 4 b-tiles
XC = GX // 128                        # 8 x-chunks (contraction)
YCS = 256                             # y-chunk size (matmul N)
YCN = GY // YCS                       # 4 y-chunks
HALF = 2048                           # x-row half width (512 x-blocks)

_cache = {}


def _build_nc():
    from concourse import bacc
    import concourse.mybir as mybir
    from concourse.tile import TileContext

    f32 = mybir.dt.float32
    f32r = mybir.dt.float32r

    nc = bacc.Bacc("TRN2", target_bir_lowering=False, debug=False,
                   enable_asserts=False, num_devices=NCORES)
    xs = nc.dram_tensor("xs", [BS, IN], f32r, kind="ExternalInput")
    e_d = [nc.dram_tensor(nm, [YCN, XC, 128, YCS], f32r, kind="ExternalInput")
           for nm in ("e0", "e1r", "e1i", "e2")]
    id_d = nc.dram_tensor("ident", [128, 128], f32r, kind="ExternalInput")
    out_d = nc.dram_tensor("out", [BS, OUT], f32, kind="ExternalOutput")

    with TileContext(nc) as tc:
        with (
            tc.tile_pool(name="consts", bufs=1) as consts,
            tc.tile_pool(name="xload", bufs=3) as xpool,
            tc.tile_pool(name="xform", bufs=2) as tf,
            tc.tile_pool(name="xt", bufs=1) as xtp,
            tc.tile_pool(name="epool", bufs=2) as ep,
            tc.tile_pool(name="outp", bufs=3) as op_,
            tc.tile_pool(name="comb", bufs=2) as cb,
            tc.tile_pool(name="tpsum", bufs=2, space="PSUM") as tps,
            tc.tile_pool(name="mpsum", bufs=1, space="PSUM") as mps,
        ):
            ident = consts.tile([128, 128], f32r)
            nc.sync.dma_start(out=ident, in_=id_d[:, :])

            # Persistent transposed+transformed activations: [x-part, xc, b]
            xt = [xtp.tile([128, XC, BS], f32r, tag=f"xt{k}", name=f"xt{k}") for k in range(4)]
            # k index: 0 -> X0, 1 -> X1r, 2 -> X1i, 3 -> X2

            # ---- Stage A: load x, forward length-4 DFT, transpose ----
            for bt in range(BT):
                for h in range(2):
                    xn = xpool.tile([128, HALF], f32r, tag="xn")
                    nc.sync.dma_start(
                        out=xn, in_=xs[bt * 128:(bt + 1) * 128,
                                       h * HALF:(h + 1) * HALF])
                    xv = xn.rearrange("p (x j) -> p x j", j=4)  # [128, 512, 4]
                    s02 = tf.tile([128, 512], f32r, tag="s02")
                    s13 = tf.tile([128, 512], f32r, tag="s13")
                    xm = [tf.tile([128, 512], f32r, tag=f"xm{k}", name=f"xm{k}") for k in range(4)]
                    nc.any.tensor_add(out=s02, in0=xv[:, :, 0], in1=xv[:, :, 2])
                    nc.any.tensor_add(out=s13, in0=xv[:, :, 1], in1=xv[:, :, 3])
                    nc.any.tensor_add(out=xm[0], in0=s02, in1=s13)   # X0
                    nc.any.tensor_sub(out=xm[3], in0=s02, in1=s13)   # X2
                    nc.any.tensor_sub(out=xm[1], in0=xv[:, :, 0], in1=xv[:, :, 2])  # X1r
                    nc.any.tensor_sub(out=xm[2], in0=xv[:, :, 3], in1=xv[:, :, 1])  # X1i
                    for k in range(4):
                        ps = tps.tile([128, 512], f32r, tag="tps")
                        for xl in range(4):
                            nc.tensor.transpose(
                                ps[:, xl * 128:(xl + 1) * 128],
                                xm[k][:, xl * 128:(xl + 1) * 128], ident)
                        # psum [x-part, (xl, b=128)] -> xt[k][:, 4h:4h+4, bt*128:...]
                        nc.any.tensor_copy(
                            out=xt[k][:, h * 4:(h + 1) * 4,
                                      bt * 128:(bt + 1) * 128],
                            in_=ps.rearrange("p (c b) -> p c b", c=4))

            # ---- Stage B: 6 matmul chains per (yc, bt), inverse DFT, store ----
            for yc in range(YCN):
                et = [ep.tile([128, XC, YCS], f32r, tag=f"e{k}", name=f"et{k}") for k in range(4)]
                for k in range(4):
                    for xc in range(XC):
                        nc.sync.dma_start(out=et[k][:, xc], in_=e_d[k][yc, xc])
                for bt in range(BT):
                    bsl = slice(bt * 128, (bt + 1) * 128)
                    y0 = mps.tile([128, YCS], f32, tag="y0")
                    y2 = mps.tile([128, YCS], f32, tag="y2")
                    p_ = mps.tile([128, YCS], f32, tag="p")   # X1r E1r
                    q_ = mps.tile([128, YCS], f32, tag="q")   # X1i E1i
                    yi = mps.tile([128, YCS], f32, tag="yi")  # X1i E1r + X1r E1i
                    for xc in range(XC):
                        st, sp = xc == 0, xc == XC - 1
                        nc.tensor.matmul(y0, xt[0][:, xc, bsl], et[0][:, xc], start=st, stop=sp)
                        nc.tensor.matmul(y2, xt[3][:, xc, bsl], et[3][:, xc], start=st, stop=sp)
                        nc.tensor.matmul(p_, xt[1][:, xc, bsl], et[1][:, xc], start=st, stop=sp)
                        nc.tensor.matmul(q_, xt[2][:, xc, bsl], et[2][:, xc], start=st, stop=sp)
                        nc.tensor.matmul(yi, xt[2][:, xc, bsl], et[1][:, xc], start=st, stop=False)
                    for xc in range(XC):
                        nc.tensor.matmul(yi, xt[1][:, xc, bsl], et[2][:, xc],
                                         start=False, stop=xc == XC - 1)
                    # inverse transform: o0=A+C o2=A-C o1=B-Yi o3=B+Yi
                    #   A=Y0+Y2, B=Y0-Y2, C=P-Q
                    # DVE/ACT may read at most ONE operand from PSUM per op:
                    # stage y0 and -q through SBUF first.
                    t_ = cb.tile([128, YCS], f32, tag="t")
                    u_ = cb.tile([128, YCS], f32, tag="u")
                    a_ = cb.tile([128, YCS], f32, tag="a")
                    b_ = cb.tile([128, YCS], f32, tag="b")
                    c_ = cb.tile([128, YCS], f32, tag="c")
                    ot = op_.tile([128, 4 * YCS], f32, tag="ot")
                    ov = ot.rearrange("p (y j) -> p y j", j=4)
                    nc.any.tensor_copy(out=t_, in_=y0)
                    nc.any.tensor_scalar_mul(u_, q_, -1.0)
                    nc.any.tensor_add(out=a_, in0=y2, in1=t_)
                    nc.any.tensor_sub(out=b_, in0=t_, in1=y2)
                    nc.any.tensor_add(out=c_, in0=p_, in1=u_)
                    nc.any.tensor_add(out=ov[:, :, 0], in0=a_, in1=c_)
                    nc.any.tensor_sub(out=ov[:, :, 2], in0=a_, in1=c_)
                    nc.any.tensor_sub(out=ov[:, :, 1], in0=b_, in1=yi)
                    nc.any.tensor_add(out=ov[:, :, 3], in0=b_, in1=yi)
                    nc.sync.dma_start(
                        out=out_d[bsl, yc * 4 * YCS:(yc + 1) * 4 * YCS], in_=ot)
    nc.compile()
    return nc


def _prep_eigens(eigens):
    """eigens (gy, gx, 4) -> four (YCN, XC, 128, YCS) f32 chunked E-matrices,
    transposed to [x, y] with irfft scale factors folded in."""
    e = np.ascontiguousarray(eigens.transpose(1, 0, 2)).astype(np.float32)  # (x, y, j)
    e0 = ((e[..., 0] + e[..., 2]) + (e[..., 1] + e[..., 3])) * 0.25
    e2 = ((e[..., 0] + e[..., 2]) - (e[..., 1] + e[..., 3])) * 0.25
    e1r = (e[..., 0] - e[..., 2]) * 0.5
    e1i = (e[..., 3] - e[..., 1]) * 0.5

    def chunk(m):  # (GX, GY) -> (YCN, XC, 128, YCS)
        return np.ascontiguousarray(
            m.reshape(XC, 128, YCN, YCS).transpose(2, 0, 1, 3))
    return chunk(e0), chunk(e1r), chunk(e1i), chunk(e2)


def kernel(x, eigens):
    from concourse.bass_utils import run_bass_kernel_spmd

    if "nc" not in _cache:
        _cache["nc"] = _build_nc()
    nc = _cache["nc"]

    x = np.ascontiguousarray(x, dtype=np.float32)
    e0, e1r, e1i, e2 = _prep_eigens(np.asarray(eigens))
    in_maps = [
        {"xs": x[c * BS:(c + 1) * BS], "e0": e0, "e1r": e1r, "e1i": e1i,
         "e2": e2, "ident": np.eye(128, dtype=np.float32)}
        for c in range(NCORES)
    ]
    res = run_bass_kernel_spmd(nc, in_maps, core_ids=list(range(NCORES)))
    return np.concatenate([r["out"] for r in res.results], axis=0)


# revision 6
# speedup vs baseline: 1.0211x; 1.0211x over previous
"""Block-circulant linear (MINI_BLOCK=4) via length-4 rFFT factorization on 8 trn2 cores.

Math: out = x @ W^T where W[4y+n, 4x+j] = eigens[y, x, (n-j) mod 4].
In the length-4 DFT domain the circulant contraction factors into 6 real
matmuls over the block-index axis gx=1024 (10.7x fewer FLOPs than dense):
  X0 = x0+x1+x2+x3, X1 = (x0-x2) + i(x3-x1), X2 = x0-x1+x2-x3  (per block of 4)
  Y0 = X0 E0, Y1 = X1*E1 (complex), Y2 = X2 E2   (contract over gx)
  o0 = Y0+2Re(Y1)+Y2, o1 = Y0-2Im(Y1)-Y2, o2 = Y0-2Re(Y1)+Y2, o3 = Y0+2Im(Y1)-Y2 (/4)

Sharding: data-parallel over batch, 512 rows per core; E-matrices (host
pre-transformed from eigens, scales folded) replicated per core. The x shard
is shipped host-transposed (pure layout) so the contraction axis lands on
SBUF partitions without any on-device transposes; the DFT butterflies are
unit-stride vector adds. Matmuls run in float32r (fp32 bits, reduced-precision
PE multiply, 4x faster than fp32; rel err ~2e-4 over K=1024).
"""
import numpy as np

B, IN, OUT, BLK = 4096, 4096, 4096, 4
GX, GY = IN // BLK, OUT // BLK        # 1024, 1024
NCORES = 8
BS = B // NCORES                      # 512 batch rows per core
BT = BS // 128                        # 4 b-tiles
XC = GX // 128                        # 8 x-chunks (contraction)
YCS = 256                             # y-chunk size (matmul N)
YCN = GY // YCS                       # 2 y-chunks

_cache = {}


def _build_nc():
    from concourse import bacc
    import concourse.mybir as mybir
    from concourse.tile import TileContext

    f32 = mybir.dt.float32
    f32r = mybir.dt.float32r

    nc = bacc.Bacc("TRN2", target_bir_lowering=False, debug=False,
                   enable_asserts=False, num_devices=NCORES)
    # x shard, transposed on host: [IN, BS] so the block axis is the DMA
    # partition axis.
    xt_d = nc.dram_tensor("xst", [IN, BS], f32r, kind="ExternalInput")
    e_d = [nc.dram_tensor(nm, [YCN, XC, 128, YCS], f32r, kind="ExternalInput")
           for nm in ("e0", "e1r", "e1i", "e2")]
    out_d = nc.dram_tensor("out", [BS, OUT], f32, kind="ExternalOutput")

    with TileContext(nc) as tc:
        with (
            tc.tile_pool(name="xload", bufs=3) as xpool,
            tc.tile_pool(name="xt", bufs=1) as xtp,
            tc.tile_pool(name="epool", bufs=2) as ep,
            tc.tile_pool(name="outp", bufs=3) as op_,
            tc.tile_pool(name="comb", bufs=2) as cb,
            tc.tile_pool(name="mpsum", bufs=1, space="PSUM") as mps,
        ):
            # Forward DFT of x, contraction-major: xt[k] is [x-part, xc, b]
            xt = [xtp.tile([128, XC, BS], f32r, tag=f"xt{k}", name=f"xt{k}")
                  for k in range(4)]  # 0 -> X0, 1 -> X1r, 2 -> X1i, 3 -> X2
            for xc in range(XC):
                xj = []
                for j in range(4):
                    t = xpool.tile([128, BS], f32r, tag=f"xj{j}", name=f"xj{j}")
                    # rows 4*(128*xc + p) + j of xst, p = 0..127
                    nc.sync.dma_start(
                        out=t,
                        in_=xt_d[:, :].rearrange("(c p j) b -> c j p b", p=128, j=4)[xc, j])
                    xj.append(t)
                s02 = xpool.tile([128, BS], f32r, tag="s02")
                s13 = xpool.tile([128, BS], f32r, tag="s13")
                nc.any.tensor_add(out=s02, in0=xj[0], in1=xj[2])
                nc.any.tensor_add(out=s13, in0=xj[1], in1=xj[3])
                nc.any.tensor_sub(out=xt[1][:, xc], in0=xj[0], in1=xj[2])
                nc.any.tensor_sub(out=xt[2][:, xc], in0=xj[3], in1=xj[1])
                nc.any.tensor_add(out=xt[0][:, xc], in0=s02, in1=s13)
                nc.any.tensor_sub(out=xt[3][:, xc], in0=s02, in1=s13)

            # Main: 6 matmul chains per (yc, bt), inverse DFT, store
            for yc in range(YCN):
                et = [ep.tile([128, XC, YCS], f32r, tag=f"e{k}", name=f"et{k}")
                      for k in range(4)]
                for k in range(4):
                    for xc in range(XC):
                        nc.sync.dma_start(out=et[k][:, xc], in_=e_d[k][yc, xc])
                for bt in range(BT):
                    bsl = slice(bt * 128, (bt + 1) * 128)
                    y0 = mps.tile([128, YCS], f32, tag="y0", bufs=2)
                    y2 = mps.tile([128, YCS], f32, tag="y2")
                    p_ = mps.tile([128, YCS], f32, tag="p", bufs=2)   # X1r E1r
                    q_ = mps.tile([128, YCS], f32, tag="q", bufs=2)   # X1i E1i
                    yi = mps.tile([128, YCS], f32, tag="yi")  # X1i E1r + X1r E1i
                    for xc in range(XC):
                        st, sp = xc == 0, xc == XC - 1
                        nc.tensor.matmul(y0, xt[0][:, xc, bsl], et[0][:, xc], start=st, stop=sp)
                        nc.tensor.matmul(y2, xt[3][:, xc, bsl], et[3][:, xc], start=st, stop=sp)
                        nc.tensor.matmul(p_, xt[1][:, xc, bsl], et[1][:, xc], start=st, stop=sp)
                        nc.tensor.matmul(q_, xt[2][:, xc, bsl], et[2][:, xc], start=st, stop=sp)
                        nc.tensor.matmul(yi, xt[2][:, xc, bsl], et[1][:, xc], start=st, stop=False)
                    for xc in range(XC):
                        nc.tensor.matmul(yi, xt[1][:, xc, bsl], et[2][:, xc],
                                         start=False, stop=xc == XC - 1)
                    # inverse DFT; DVE/ACT may read only ONE PSUM operand per
                    # op, so stage y0 and -q through SBUF.
                    t_ = cb.tile([128, YCS], f32, tag="t")
                    u_ = cb.tile([128, YCS], f32, tag="u")
                    a_ = cb.tile([128, YCS], f32, tag="a")
                    b_ = cb.tile([128, YCS], f32, tag="b")
                    c_ = cb.tile([128, YCS], f32, tag="c")
                    ot = op_.tile([128, 4 * YCS], f32, tag="ot")
                    ov = ot.rearrange("p (y j) -> p y j", j=4)
                    nc.any.tensor_copy(out=t_, in_=y0)
                    nc.any.tensor_scalar_mul(u_, q_, -1.0)
                    nc.any.tensor_add(out=a_, in0=y2, in1=t_)   # Y0+Y2
                    nc.any.tensor_sub(out=b_, in0=t_, in1=y2)   # Y0-Y2
                    nc.any.tensor_add(out=c_, in0=p_, in1=u_)   # Y1r = P-Q
                    nc.any.tensor_add(out=ov[:, :, 0], in0=a_, in1=c_)
                    nc.any.tensor_sub(out=ov[:, :, 2], in0=a_, in1=c_)
                    nc.any.tensor_sub(out=ov[:, :, 1], in0=b_, in1=yi)
                    nc.any.tensor_add(out=ov[:, :, 3], in0=b_, in1=yi)
                    nc.sync.dma_start(
                        out=out_d[bsl, yc * 4 * YCS:(yc + 1) * 4 * YCS], in_=ot)
    nc.compile()
    return nc


def _prep_eigens(eigens):
    """eigens (gy, gx, 4) -> four (YCN, XC, 128, YCS) f32 chunked E-matrices,
    transposed to [x, y] with irfft scale factors folded in."""
    e = np.ascontiguousarray(eigens.transpose(1, 0, 2)).astype(np.float32)  # (x, y, j)
    e0 = ((e[..., 0] + e[..., 2]) + (e[..., 1] + e[..., 3])) * 0.25
    e2 = ((e[..., 0] + e[..., 2]) - (e[..., 1] + e[..., 3])) * 0.25
    e1r = (e[..., 0] - e[..., 2]) * 0.5
    e1i = (e[..., 3] - e[..., 1]) * 0.5

    def chunk(m):  # (GX, GY) -> (YCN, XC, 128, YCS)
        return np.ascontiguousarray(
            m.reshape(XC, 128, YCN, YCS).transpose(2, 0, 1, 3))
    return chunk(e0), chunk(e1r), chunk(e1i), chunk(e2)


def _in_maps(x, eigens):
    x = np.ascontiguousarray(x, dtype=np.float32)
    e0, e1r, e1i, e2 = _prep_eigens(np.asarray(eigens))
    xT = np.ascontiguousarray(x.T)  # [IN, B]
    return [
        {"xst": np.ascontiguousarray(xT[:, c * BS:(c + 1) * BS]),
         "e0": e0, "e1r": e1r, "e1i": e1i, "e2": e2}
        for c in range(NCORES)
    ]


def kernel(x, eigens):
    from concourse.bass_utils import run_bass_kernel_spmd

    if "nc" not in _cache:
        _cache["nc"] = _build_nc()
    res = run_bass_kernel_spmd(_cache["nc"], _in_maps(x, eigens),
                               core_ids=list(range(NCORES)))
    return np.concatenate([r["out"] for r in res.results], axis=0)


# revision 7
# speedup vs baseline: 1.0493x; 1.0276x over previous
"""Block-circulant linear (MINI_BLOCK=4) via length-4 rFFT factorization on 8 trn2 cores.

Math: out = x @ W^T where W[4y+n, 4x+j] = eigens[y, x, (n-j) mod 4].
In the length-4 DFT domain the circulant contraction factors into 6 real
matmuls over the block-index axis gx=1024 (10.7x fewer FLOPs than dense):
  X0 = x0+x1+x2+x3, X1 = (x0-x2) + i(x3-x1), X2 = x0-x1+x2-x3  (per block of 4)
  Y0 = X0 E0, Y1 = X1*E1 (complex), Y2 = X2 E2   (contract over gx)
  o0 = Y0+2Re(Y1)+Y2, o1 = Y0-2Im(Y1)-Y2, o2 = Y0-2Re(Y1)+Y2, o3 = Y0+2Im(Y1)-Y2 (/4)

Sharding: data-parallel over batch, 512 rows per core; E-matrices (host
pre-transformed from eigens, scales folded) replicated per core. The x shard
is shipped host-transposed (pure layout) so the contraction axis lands on
SBUF partitions without any on-device transposes; the DFT butterflies are
unit-stride vector adds. Matmuls run in float32r (fp32 bits, reduced-precision
PE multiply, 4x faster than fp32; rel err ~2e-4 over K=1024).
"""
import numpy as np

B, IN, OUT, BLK = 4096, 4096, 4096, 4
GX, GY = IN // BLK, OUT // BLK        # 1024, 1024
NCORES = 8
BS = B // NCORES                      # 512 batch rows per core
BT = BS // 128                        # 4 b-tiles
XC = GX // 128                        # 8 x-chunks (contraction)
YCS = 256                             # y-chunk size (matmul N)
YCN = GY // YCS                       # 2 y-chunks

_cache = {}


def _build_nc():
    from concourse import bacc
    import concourse.mybir as mybir
    from concourse.tile import TileContext

    f32 = mybir.dt.float32
    f32r = mybir.dt.float32r

    nc = bacc.Bacc("TRN2", target_bir_lowering=False, debug=False,
                   enable_asserts=False, num_devices=NCORES)
    # x shard, transposed on host: [IN, BS] so the block axis is the DMA
    # partition axis.
    xt_d = nc.dram_tensor("xst", [IN, BS], f32r, kind="ExternalInput")
    e_d = [nc.dram_tensor(nm, [YCN, XC, 128, YCS], f32r, kind="ExternalInput")
           for nm in ("e0", "e1r", "e1i", "e2")]
    out_d = nc.dram_tensor("out", [BS, OUT], f32, kind="ExternalOutput")

    with TileContext(nc) as tc:
        with (
            tc.tile_pool(name="xload", bufs=3) as xpool,
            tc.tile_pool(name="xt", bufs=1) as xtp,
            tc.tile_pool(name="epool", bufs=2) as ep,
            tc.tile_pool(name="outp", bufs=3) as op_,
            tc.tile_pool(name="comb", bufs=2) as cb,
            tc.tile_pool(name="mpsum", bufs=1, space="PSUM") as mps,
        ):
            # Forward DFT of x, contraction-major: xt[k] is [x-part, xc, b].
            # yc=0's E chunks are loaded interleaved per-xc with the x loads
            # so the first matmul chain can start after ~1.5 MB of DMA.
            xt = [xtp.tile([128, XC, BS], f32r, tag=f"xt{k}", name=f"xt{k}")
                  for k in range(4)]  # 0 -> X0, 1 -> X1r, 2 -> X1i, 3 -> X2
            et0 = [ep.tile([128, XC, YCS], f32r, tag=f"e{k}", name=f"et{k}")
                   for k in range(4)]
            for xc in range(XC):
                xj = []
                for j in range(4):
                    t = xpool.tile([128, BS], f32r, tag=f"xj{j}", name=f"xj{j}")
                    # rows 4*(128*xc + p) + j of xst, p = 0..127
                    nc.sync.dma_start(
                        out=t,
                        in_=xt_d[:, :].rearrange("(c p j) b -> c j p b", p=128, j=4)[xc, j])
                    xj.append(t)
                for k in range(4):
                    nc.sync.dma_start(out=et0[k][:, xc], in_=e_d[k][0, xc])
                s02 = xpool.tile([128, BS], f32r, tag="s02")
                s13 = xpool.tile([128, BS], f32r, tag="s13")
                nc.vector.tensor_add(out=s02, in0=xj[0], in1=xj[2])
                nc.vector.tensor_add(out=s13, in0=xj[1], in1=xj[3])
                nc.vector.tensor_sub(out=xt[1][:, xc], in0=xj[0], in1=xj[2])
                nc.vector.tensor_sub(out=xt[2][:, xc], in0=xj[3], in1=xj[1])
                nc.vector.tensor_add(out=xt[0][:, xc], in0=s02, in1=s13)
                nc.vector.tensor_sub(out=xt[3][:, xc], in0=s02, in1=s13)

            # Main: 6 matmul chains per (yc, bt), inverse DFT, store
            for yc in range(YCN):
                if yc == 0:
                    et = et0
                else:
                    et = [ep.tile([128, XC, YCS], f32r, tag=f"e{k}", name=f"et{k}")
                          for k in range(4)]
                    for k in range(4):
                        for xc in range(XC):
                            nc.sync.dma_start(out=et[k][:, xc], in_=e_d[k][yc, xc])
                for bt in range(BT):
                    bsl = slice(bt * 128, (bt + 1) * 128)
                    y0 = mps.tile([128, YCS], f32, tag="y0")
                    y2 = mps.tile([128, YCS], f32, tag="y2")
                    p_ = mps.tile([128, YCS], f32, tag="p", bufs=2)   # X1r E1r
                    q_ = mps.tile([128, YCS], f32, tag="q", bufs=2)   # X1i E1i
                    yi = mps.tile([128, YCS], f32, tag="yi", bufs=2)  # X1i E1r + X1r E1i
                    # yi (the longest chain, freed mid-combine) runs first so
                    # its bank recycles a full chain-length ahead of reuse.
                    for xc in range(XC):
                        nc.tensor.matmul(yi, xt[2][:, xc, bsl], et[1][:, xc],
                                         start=xc == 0, stop=False)
                    for xc in range(XC):
                        nc.tensor.matmul(yi, xt[1][:, xc, bsl], et[2][:, xc],
                                         start=False, stop=xc == XC - 1)
                    for xc in range(XC):
                        st, sp = xc == 0, xc == XC - 1
                        nc.tensor.matmul(y0, xt[0][:, xc, bsl], et[0][:, xc], start=st, stop=sp)
                        nc.tensor.matmul(y2, xt[3][:, xc, bsl], et[3][:, xc], start=st, stop=sp)
                        nc.tensor.matmul(p_, xt[1][:, xc, bsl], et[1][:, xc], start=st, stop=sp)
                        nc.tensor.matmul(q_, xt[2][:, xc, bsl], et[2][:, xc], start=st, stop=sp)
                    # inverse DFT, ops ordered to free PSUM banks in chain
                    # order; DVE/ACT read at most ONE PSUM operand per op.
                    t_ = cb.tile([128, YCS], f32, tag="t")
                    u_ = cb.tile([128, YCS], f32, tag="u")
                    a_ = cb.tile([128, YCS], f32, tag="a")
                    b_ = cb.tile([128, YCS], f32, tag="b")
                    c_ = cb.tile([128, YCS], f32, tag="c")
                    ot = op_.tile([128, 4 * YCS], f32, tag="ot")
                    ov = ot.rearrange("p (y j) -> p y j", j=4)
                    nc.scalar.copy(out=t_, in_=y0)               # frees y0
                    nc.vector.tensor_sub(out=b_, in0=t_, in1=y2) # Y0-Y2
                    nc.vector.tensor_add(out=a_, in0=y2, in1=t_) # Y0+Y2, frees y2
                    nc.vector.tensor_sub(out=ov[:, :, 1], in0=b_, in1=yi)
                    nc.vector.tensor_add(out=ov[:, :, 3], in0=b_, in1=yi)  # frees yi
                    nc.scalar.mul(u_, q_, -1.0)                  # frees q
                    nc.vector.tensor_add(out=c_, in0=p_, in1=u_) # Y1r = P-Q, frees p
                    nc.vector.tensor_add(out=ov[:, :, 0], in0=a_, in1=c_)
                    nc.vector.tensor_sub(out=ov[:, :, 2], in0=a_, in1=c_)
                    nc.sync.dma_start(
                        out=out_d[bsl, yc * 4 * YCS:(yc + 1) * 4 * YCS], in_=ot)
    nc.compile()
    return nc


def _prep_eigens(eigens):
    """eigens (gy, gx, 4) -> four (YCN, XC, 128, YCS) f32 chunked E-matrices,
    transposed to [x, y] with irfft scale factors folded in."""
    e = np.ascontiguousarray(eigens.transpose(1, 0, 2)).astype(np.float32)  # (x, y, j)
    e0 = ((e[..., 0] + e[..., 2]) + (e[..., 1] + e[..., 3])) * 0.25
    e2 = ((e[..., 0] + e[..., 2]) - (e[..., 1] + e[..., 3])) * 0.25
    e1r = (e[..., 0] - e[..., 2]) * 0.5
    e1i = (e[..., 3] - e[..., 1]) * 0.5

    def chunk(m):  # (GX, GY) -> (YCN, XC, 128, YCS)
        return np.ascontiguousarray(
            m.reshape(XC, 128, YCN, YCS).transpose(2, 0, 1, 3))
    return chunk(e0), chunk(e1r), chunk(e1i), chunk(e2)


def _in_maps(x, eigens):
    x = np.ascontiguousarray(x, dtype=np.float32)
    e0, e1r, e1i, e2 = _prep_eigens(np.asarray(eigens))
    xT = np.ascontiguousarray(x.T)  # [IN, B]
    return [
        {"xst": np.ascontiguousarray(xT[:, c * BS:(c + 1) * BS]),
         "e0": e0, "e1r": e1r, "e1i": e1i, "e2": e2}
        for c in range(NCORES)
    ]


def kernel(x, eigens):
    from concourse.bass_utils import run_bass_kernel_spmd

    if "nc" not in _cache:
        _cache["nc"] = _build_nc()
    res = run_bass_kernel_spmd(_cache["nc"], _in_maps(x, eigens),
                               core_ids=list(range(NCORES)))
    return np.concatenate([r["out"] for r in res.results], axis=0)
